# revision 1
# baseline (speedup 1.0000x reference)
"""Trainium2 Bass kernel for nn_AdaptiveGatingHybridActivation.

Data-parallel across 8 NeuronCores: each core processes 256 of the 2048
(batch*seq) rows, full vocab (V=50257). All vocab reductions are local to a
core; the final scalar mean is assembled on the host from per-row losses.

Math (per row, x = logits row, m/sigma = row mean / unbiased std):
  e   = exp(x / (1 + 0.1*sigma))                 Z    = sum(e)
  u   = (x - m) / (sigma + 1e-10)
  T   = tanh(u/2)              (gate g = 0.5 + 0.5*T)
  s   = 1 + log1p(relu(x - m))
  th  = tanh(x / s)            (relu_probs r = 0.5 + 0.5*th)
  lg  = ln(g + 1e-10) = Ln(0.5*T + 0.5)
  combined c = A*g*r + (1 - A*g)*e/Z,  A = 0.5
  S_c = A*(0.25*V + 0.25*(ST + Sth + STth)) ... via sums of T, th, e, lg and
        the products T*e, T*th, T*lg (tanh substitution for sigmoid keeps
        every transcendental in the {exp, ln} + {tanh} ACT table sets).
  loss_row = -ln(clip(c_t/(S_c+eps), eps, 1)) + 0.01*(0.5*Slg + 0.5*STlg)

Engine plan: ACT does the 5 transcendental passes (with fused accum_out row
sums), DVE does relu/reciprocal/products (tensor_tensor_reduce fuses
product+row-sum), fp16 intermediates for 2x/4x DVE modes. x is read once as
f32 (stats pass, converted to fp16 and stashed to DRAM), then re-read as fp16
for the compute pass. ACT instructions are chained in table-set groups
({exp,ln} then {tanh}) to minimize ~2.7us ACT table loads.
"""

import numpy as np

import concourse.bass as bass
import concourse.tile as tile
from concourse import mybir
from concourse.tile import add_dep_helper



def _split_multi_waits(nc):
    """This walrus build rejects instructions carrying more than one sync
    wait. Hoist extra waits onto same-engine no-ops placed just before."""
    n_split = [0]
    for fn in nc.m.functions:
        for bb in fn.blocks:
            out = []
            for inst in bb.instructions:
                si = inst.sync_info
                waits = list(si.on_wait) if (si is not None and si.on_wait) else []
                if len(waits) > 1:
                    for w in waits[:-1]:
                        n_split[0] += 1
                        nop = mybir.InstNoOp(
                            name=f"waitsplit_{n_split[0]}",
                            engine=inst.engine,
                            bass_nofuse=True,
                        )
                        nop.sync_info = mybir.SyncInfo(on_wait=[w], on_update=[])
                        out.append(nop)
                    inst.sync_info = mybir.SyncInfo(
                        on_wait=[waits[-1]], on_update=list(si.on_update or []))
                out.append(inst)
            bb.instructions[:] = out
    return n_split[0]


F32 = mybir.dt.float32
F16 = mybir.dt.float16
ALU = mybir.AluOpType
ACTF = mybir.ActivationFunctionType

V = 50257
B, S = 4, 512
NROWS = B * S            # 2048
NCORES = 8
ROWS_PER_CORE = NROWS // NCORES   # 256
P = 128                  # partitions
NT = ROWS_PER_CORE // P  # 2 row-tiles per core
F = 2048                 # vocab chunk (free dim)
NCHUNK = (V + F - 1) // F          # 25
CHUNKS = [(j * F, min(F, V - j * F)) for j in range(NCHUNK)]
G = 4                    # chunks per ACT table-set phase group

ALPHA = 0.5
BETA = 0.1
EPS = 1e-10
E_CONST = float(np.e)


def _groups():
    out = []
    for g0 in range(0, NCHUNK, G):
        out.append(list(range(g0, min(g0 + G, NCHUNK))))
    return out


def build_kernel(tc, x, xt, x16, out):
    nc = tc.nc

    act_chain = [None]

    def chain(instr):
        # Serialize ACT engine in issue order so activations stay grouped by
        # table set (scheduler is otherwise free to interleave exp/ln/tanh).
        if act_chain[0] is not None:
            add_dep_helper(instr.ins, act_chain[0].ins, False,
                           "ACT table-set ordering")
        act_chain[0] = instr
        return instr

    from contextlib import ExitStack
    with ExitStack() as ctx:
        stg = ctx.enter_context(tc.tile_pool(name="stg", bufs=2))
        cvt = ctx.enter_context(tc.tile_pool(name="cvt", bufs=2))
        xcp = ctx.enter_context(tc.tile_pool(name="xcp", bufs=2 * G + 1))
        ep = ctx.enter_context(tc.tile_pool(name="ep", bufs=G + 3))
        Tp = ctx.enter_context(tc.tile_pool(name="Tp", bufs=G + 4))
        t2p = ctx.enter_context(tc.tile_pool(name="t2p", bufs=G + 3))
        wp = ctx.enter_context(tc.tile_pool(name="wp", bufs=2))
        sp_ = ctx.enter_context(tc.tile_pool(name="sp", bufs=2))
        rcp = ctx.enter_context(tc.tile_pool(name="rcp", bufs=4))
        qp = ctx.enter_context(tc.tile_pool(name="qp", bufs=2))
        thp = ctx.enter_context(tc.tile_pool(name="thp", bufs=2))
        lgp = ctx.enter_context(tc.tile_pool(name="lgp", bufs=2))
        sing = ctx.enter_context(tc.tile_pool(name="sing", bufs=1))

        scr1 = sing.tile([P, 1], F16, tag="scr1", name="scr1")

        def scr(cs):
            return scr1.broadcast_to((P, cs))

        # persistent per-row stats, one column per row-tile
        def s2(tag):
            return sing.tile([P, NT], F32, tag=tag, name=tag)

        m2, var2, sig2 = s2("m2"), s2("var2"), s2("sig2")
        invt2, istd22, nb22 = s2("invt2"), s2("istd22"), s2("nb22")
        QN = ["Z", "ST", "Sth", "Slg", "STe", "STth", "STlg"]
        sums = {q: s2("sum_" + q) for q in QN}
        Sx2, Sxx2 = s2("Sx2"), s2("Sxx2")

        cE = sing.tile([P, 1], F32, tag="cE", name="cE")
        nc.vector.memset(cE, E_CONST)
        cHalf = sing.tile([P, 1], F32, tag="cHalf", name="cHalf")
        nc.vector.memset(cHalf, 0.5)

        partials = {}
        for t in range(NT):
            for q in QN + ["Sx", "Sxx"]:
                partials[(q, t)] = sing.tile([P, NCHUNK], F32, tag=f"p_{q}_{t}", name=f"p_{q}_{t}")

        def pass1(t):
            for j, (c0, cs) in enumerate(CHUNKS):
                sg = stg.tile([P, F], F32, tag="stg")
                nc.default_dma_engine.dma_start(
                    out=sg[:, :cs], in_=x[t, :, c0:c0 + cs])
                cv = cvt.tile([P, F], F16, tag="cvt")
                # convert f32 -> fp16, fused row-sum(x)
                nc.vector.tensor_scalar(
                    out=cv[:, :cs], in0=sg[:, :cs], scalar1=1.0, scalar2=0.0,
                    op0=ALU.mult, op1=ALU.add,
                    accum_out=partials[("Sx", t)][:, j:j + 1])
                # sum(x^2), product+reduce fused; out dumped to PSUM scratch
                nc.vector.scalar_tensor_tensor(
                    out=scr(cs), in0=cv[:, :cs], scalar=1.0, in1=cv[:, :cs],
                    op0=ALU.mult, op1=ALU.mult,
                    accum_out=partials[("Sxx", t)][:, j:j + 1])
                nc.default_dma_engine.dma_start(
                    out=x16[t, :, c0:c0 + cs], in_=cv[:, :cs])

        def stats(t):
            ts = slice(t, t + 1)
            nc.vector.tensor_reduce(
                out=Sx2[:, ts], in_=partials[("Sx", t)], axis=mybir.AxisListType.X,
                op=ALU.add)
            nc.vector.tensor_reduce(
                out=Sxx2[:, ts], in_=partials[("Sxx", t)], axis=mybir.AxisListType.X,
                op=ALU.add)
            nc.vector.tensor_scalar(
                out=m2[:, ts], in0=Sx2[:, ts], scalar1=1.0 / V, scalar2=None,
                op0=ALU.mult)
            # var = (Sxx - Sx*m) / (V-1)  [unbiased]
            nc.vector.scalar_tensor_tensor(
                out=var2[:, ts], in0=Sx2[:, ts], scalar=m2[:, ts],
                in1=Sxx2[:, ts], op0=ALU.mult, op1=ALU.subtract)
            # now var2 = Sx*m - Sxx  -> * (-1/(V-1))
            nc.vector.tensor_scalar(
                out=var2[:, ts], in0=var2[:, ts], scalar1=-1.0 / (V - 1),
                scalar2=None, op0=ALU.mult)
            chain(nc.scalar.activation(
                out=sig2[:, ts], in_=var2[:, ts], func=ACTF.Sqrt))
            # invt = 1/(1 + 0.1*sigma)
            nc.vector.tensor_scalar(
                out=invt2[:, ts], in0=sig2[:, ts], scalar1=BETA, scalar2=1.0,
                op0=ALU.mult, op1=ALU.add)
            nc.vector.reciprocal(out=invt2[:, ts], in_=invt2[:, ts])
            # istd2 = 1/(2*sigma + 2e-10)
            nc.vector.tensor_scalar(
                out=istd22[:, ts], in0=sig2[:, ts], scalar1=2.0, scalar2=2.0 * EPS,
                op0=ALU.mult, op1=ALU.add)
            nc.vector.reciprocal(out=istd22[:, ts], in_=istd22[:, ts])
            # nb2 = -m * istd2
            nc.vector.tensor_scalar(
                out=nb22[:, ts], in0=m2[:, ts], scalar1=istd22[:, ts],
                scalar2=-1.0, op0=ALU.mult, op1=ALU.mult)

        def passC(t):
            ts = slice(t, t + 1)
            pend = []   # (j, T_tile) awaiting lg/T*lg in next LN phase

            def ln_phase(jlist, prev):
                tiles = {}
                for j in jlist:
                    c0, cs = CHUNKS[j]
                    xc = xcp.tile([P, F], F16, tag="xc")
                    nc.default_dma_engine.dma_start(
                        out=xc[:, :cs], in_=x16[t, :, c0:c0 + cs])
                    tiles[j] = {"xc": xc, "cs": cs}
                for j in jlist:
                    d = tiles[j]
                    e = ep.tile([P, F], F16, tag="e")
                    chain(nc.scalar.activation(
                        out=e[:, :d["cs"]], in_=d["xc"][:, :d["cs"]],
                        func=ACTF.Exp, scale=invt2[:, ts],
                        accum_out=partials[("Z", t)][:, j:j + 1]))
                    d["e"] = e
                for j in jlist:
                    d = tiles[j]
                    w = wp.tile([P, F], F16, tag="w")
                    nc.vector.tensor_scalar(
                        out=w[:, :d["cs"]], in0=d["xc"][:, :d["cs"]],
                        scalar1=m2[:, ts], scalar2=0.0,
                        op0=ALU.subtract, op1=ALU.max)
                    # s' = ln(relu(x-m)+1) + 1 = Ln(e*w + e)
                    s = sp_.tile([P, F], F16, tag="s")
                    chain(nc.scalar.activation(
                        out=s[:, :d["cs"]], in_=w[:, :d["cs"]],
                        func=ACTF.Ln, scale=E_CONST, bias=cE))
                    d["s"] = s
                for (j, Tprev, csp) in prev:
                    lg = lgp.tile([P, F], F16, tag="lg")
                    chain(nc.scalar.activation(
                        out=lg[:, :csp], in_=Tprev[:, :csp],
                        func=ACTF.Ln, scale=0.5, bias=cHalf,
                        accum_out=partials[("Slg", t)][:, j:j + 1]))
                    nc.vector.scalar_tensor_tensor(
                        out=scr(csp), in0=Tprev[:, :csp], scalar=1.0,
                        in1=lg[:, :csp], op0=ALU.mult, op1=ALU.mult,
                        accum_out=partials[("STlg", t)][:, j:j + 1])
                for j in jlist:
                    # rc = 1/s' via fp16 Newton: relative-minimax linear seed
                    # on s' in [1, 3.2], two NR steps (rel err ~7e-4).
                    d = tiles[j]
                    cs = d["cs"]
                    s = d["s"]
                    rc0 = rcp.tile([P, F], F16, tag="rc")
                    nc.vector.tensor_scalar(
                        out=rc0[:, :cs], in0=s[:, :cs], scalar1=-0.2628,
                        scalar2=1.1038, op0=ALU.mult, op1=ALU.add)
                    q = qp.tile([P, F], F16, tag="q")
                    nc.vector.tensor_mul(out=q[:, :cs], in0=s[:, :cs],
                                         in1=rc0[:, :cs])
                    nc.vector.tensor_scalar(
                        out=q[:, :cs], in0=q[:, :cs], scalar1=-1.0,
                        scalar2=2.0, op0=ALU.mult, op1=ALU.add)
                    rc1 = rcp.tile([P, F], F16, tag="rc")
                    nc.vector.tensor_mul(out=rc1[:, :cs], in0=rc0[:, :cs],
                                         in1=q[:, :cs])
                    q2 = qp.tile([P, F], F16, tag="q")
                    nc.vector.tensor_mul(out=q2[:, :cs], in0=s[:, :cs],
                                         in1=rc1[:, :cs])
                    nc.vector.tensor_scalar(
                        out=q2[:, :cs], in0=q2[:, :cs], scalar1=-1.0,
                        scalar2=2.0, op0=ALU.mult, op1=ALU.add)
                    rc2 = rcp.tile([P, F], F16, tag="rc")
                    nc.vector.tensor_mul(out=rc2[:, :cs], in0=rc1[:, :cs],
                                         in1=q2[:, :cs])
                    t2 = t2p.tile([P, F], F16, tag="t2")
                    nc.vector.tensor_mul(
                        out=t2[:, :cs], in0=d["xc"][:, :cs], in1=rc2[:, :cs])
                    d["t2"] = t2
                return tiles

            def tanh_phase(jlist, tiles):
                nxt = []
                for j in jlist:
                    d = tiles[j]
                    cs = d["cs"]
                    T = Tp.tile([P, F], F16, tag="T")
                    chain(nc.scalar.activation(
                        out=T[:, :cs], in_=d["xc"][:, :cs], func=ACTF.Tanh,
                        scale=istd22[:, ts], bias=nb22[:, ts],
                        accum_out=partials[("ST", t)][:, j:j + 1]))
                    th = thp.tile([P, F], F16, tag="th")
                    chain(nc.scalar.activation(
                        out=th[:, :cs], in_=d["t2"][:, :cs], func=ACTF.Tanh,
                        accum_out=partials[("Sth", t)][:, j:j + 1]))
                    nc.vector.scalar_tensor_tensor(
                        out=scr(cs), in0=T[:, :cs], scalar=1.0,
                        in1=d["e"][:, :cs], op0=ALU.mult, op1=ALU.mult,
                        accum_out=partials[("STe", t)][:, j:j + 1])
                    nc.vector.scalar_tensor_tensor(
                        out=scr(cs), in0=T[:, :cs], scalar=1.0,
                        in1=th[:, :cs], op0=ALU.mult, op1=ALU.mult,
                        accum_out=partials[("STth", t)][:, j:j + 1])
                    nxt.append((j, T, cs))
                return nxt

            for jlist in _groups():
                tiles = ln_phase(jlist, pend)
                pend = tanh_phase(jlist, tiles)
            return pend   # last group's T tiles: lg handled in finalize

        def flush_lg(t, pend):
            for (j, Tprev, csp) in pend:
                lg = lgp.tile([P, F], F16, tag="lg")
                chain(nc.scalar.activation(
                    out=lg[:, :csp], in_=Tprev[:, :csp],
                    func=ACTF.Ln, scale=0.5, bias=cHalf,
                    accum_out=partials[("Slg", t)][:, j:j + 1]))
                nc.vector.scalar_tensor_tensor(
                    out=scr(csp), in0=Tprev[:, :csp], scalar=1.0,
                    in1=lg[:, :csp], op0=ALU.mult, op1=ALU.mult,
                    accum_out=partials[("STlg", t)][:, j:j + 1])

        def finalize():
            # reduce partials -> per-row sums
            for t in range(NT):
                for q in QN:
                    nc.vector.tensor_reduce(
                        out=sums[q][:, t:t + 1], in_=partials[(q, t)],
                        axis=mybir.AxisListType.X, op=ALU.add)

            def tmp(tag):
                return sing.tile([P, NT], F32, tag=tag, name=tag)

            xts = tmp("xts")
            nc.default_dma_engine.dma_start(out=xts, in_=xt)

            Z, ST, Sth = sums["Z"], sums["ST"], sums["Sth"]
            Slg, STe, STth, STlg = (sums["Slg"], sums["STe"], sums["STth"],
                                    sums["STlg"])
            a1, rZ, q1, Sc = tmp("a1"), tmp("rZ"), tmp("q1"), tmp("Sc")
            nc.vector.tensor_add(out=a1, in0=ST, in1=Sth)
            nc.vector.tensor_add(out=a1, in0=a1, in1=STth)
            nc.vector.reciprocal(out=rZ, in_=Z)
            ge2 = tmp("ge2")
            nc.vector.tensor_add(out=ge2, in0=Z, in1=STe)
            nc.vector.tensor_mul(out=q1, in0=ge2, in1=rZ)
            # Sc = 0.125*V + 1 + 0.125*a1 - 0.25*q1
            s1 = tmp("s1")
            nc.vector.tensor_scalar(
                out=s1, in0=a1, scalar1=0.125, scalar2=0.125 * V + 1.0,
                op0=ALU.mult, op1=ALU.add)
            nc.vector.scalar_tensor_tensor(
                out=Sc, in0=q1, scalar=-0.25, in1=s1, op0=ALU.mult, op1=ALU.add)
            # CE pieces from gathered target logits
            v1t, et = tmp("v1t"), tmp("et")
            nc.vector.tensor_mul(out=v1t, in0=xts, in1=invt2)
            chain(nc.scalar.activation(out=et, in_=v1t, func=ACTF.Exp))
            wt, st_ = tmp("wt"), tmp("st_")
            nc.vector.tensor_sub(out=wt, in0=xts, in1=m2)
            nc.vector.tensor_scalar(
                out=wt, in0=wt, scalar1=0.0, scalar2=None, op0=ALU.max)
            chain(nc.scalar.activation(
                out=st_, in_=wt, func=ACTF.Ln, scale=E_CONST, bias=cE))
            rct, t2t = tmp("rct"), tmp("t2t")
            nc.vector.reciprocal(out=rct, in_=st_)
            nc.vector.tensor_mul(out=t2t, in0=xts, in1=rct)
            u1 = tmp("u1")
            nc.vector.tensor_mul(out=u1, in0=xts, in1=istd22)
            nc.vector.tensor_add(out=u1, in0=u1, in1=nb22)
            Tt, tht = tmp("Tt"), tmp("tht")
            chain(nc.scalar.activation(out=Tt, in_=u1, func=ACTF.Tanh))
            chain(nc.scalar.activation(out=tht, in_=t2t, func=ACTF.Tanh))
            gt, rt = tmp("gt"), tmp("rt")
            nc.vector.tensor_scalar(
                out=gt, in0=Tt, scalar1=0.5, scalar2=0.5, op0=ALU.mult,
                op1=ALU.add)
            nc.vector.tensor_scalar(
                out=rt, in0=tht, scalar1=0.5, scalar2=0.5, op0=ALU.mult,
                op1=ALU.add)
            erz, p1, p2, c1, ct = tmp("erz"), tmp("p1"), tmp("p2"), tmp("c1"), tmp("ct")
            nc.vector.tensor_mul(out=erz, in0=et, in1=rZ)
            nc.vector.tensor_mul(out=p1, in0=gt, in1=rt)
            nc.vector.tensor_mul(out=p2, in0=gt, in1=erz)
            nc.vector.scalar_tensor_tensor(
                out=c1, in0=p1, scalar=0.5, in1=erz, op0=ALU.mult, op1=ALU.add)
            nc.vector.scalar_tensor_tensor(
                out=ct, in0=p2, scalar=-0.5, in1=c1, op0=ALU.mult, op1=ALU.add)
            scd, rsc, pt = tmp("scd"), tmp("rsc"), tmp("pt")
            nc.vector.tensor_scalar(
                out=scd, in0=Sc, scalar1=EPS, scalar2=None, op0=ALU.add)
            nc.vector.reciprocal(out=rsc, in_=scd)
            nc.vector.tensor_mul(out=pt, in0=ct, in1=rsc)
            nc.vector.tensor_scalar(
                out=pt, in0=pt, scalar1=EPS, scalar2=1.0, op0=ALU.max,
                op1=ALU.min)
            lnp = tmp("lnp")
            chain(nc.scalar.activation(out=lnp, in_=pt, func=ACTF.Ln))
            # loss = -lnp + 0.005*(Slg + STlg)
            sgl = tmp("sgl")
            nc.vector.tensor_add(out=sgl, in0=Slg, in1=STlg)
            loss = tmp("loss")
            nc.vector.scalar_tensor_tensor(
                out=loss, in0=sgl, scalar=0.005, in1=lnp, op0=ALU.mult,
                op1=ALU.subtract)
            nc.default_dma_engine.dma_start(out=out, in_=loss)

        pass1(0)
        stats(0)
        pass1(1)
        pend0 = passC(0)
        flush_lg(0, pend0)
        stats(1)
        pend1 = passC(1)
        flush_lg(1, pend1)
        finalize()


def build_nc(split_waits=True):
    nc = bass.Bass("TRN2", debug=False, target_bir_lowering=False,
                   num_devices=NCORES)
    x = nc.dram_tensor("x", [NT, P, V], F32, kind="ExternalInput").ap()
    xt = nc.dram_tensor("xt", [P, NT], F32, kind="ExternalInput").ap()
    x16 = nc.dram_tensor("x16", [NT, P, V], F16).ap()
    out = nc.dram_tensor("out", [P, NT], F32, kind="ExternalOutput").ap()
    with tile.TileContext(nc) as tc:
        build_kernel(tc, x, xt, x16, out)
    if split_waits:
        _split_multi_waits(nc)
    return nc


_NC_CACHE = None


def _get_nc():
    global _NC_CACHE
    if _NC_CACHE is None:
        _NC_CACHE = build_nc()
    return _NC_CACHE


def make_in_maps(logits, targets):
    lg = np.ascontiguousarray(np.asarray(logits, dtype=np.float32)).reshape(
        NROWS, V)
    tg = np.asarray(targets).reshape(NROWS).astype(np.int64)
    xt_rows = lg[np.arange(NROWS), tg].astype(np.float32)
    in_maps = []
    for c in range(NCORES):
        r0 = c * ROWS_PER_CORE
        x_c = lg[r0:r0 + ROWS_PER_CORE].reshape(NT, P, V)
        xt_c = np.ascontiguousarray(
            xt_rows[r0:r0 + ROWS_PER_CORE].reshape(NT, P).T)
        in_maps.append({"x": x_c, "xt": xt_c})
    return in_maps


def kernel(logits, targets):
    from concourse.bass_utils import run_bass_kernel_spmd
    nc = _get_nc()
    in_maps = make_in_maps(logits, targets)
    res = run_bass_kernel_spmd(nc, in_maps, core_ids=list(range(NCORES)))
    rows = np.concatenate(
        [res.results[c]["out"].T.reshape(ROWS_PER_CORE) for c in range(NCORES)])
    return np.asarray(rows.mean(), dtype=np.float32)



# revision 2
# speedup vs baseline: 2.3837x; 2.3837x over previous
"""Trainium2 Bass kernel for nn_AdaptiveGatingHybridActivation.

Data-parallel across 8 NeuronCores: each core does 256 rows (2 tiles of 128
partitions) x V=50257 vocab. Per-row math (x row, m/sigma mean/unbiased-std):
  e  = exp(x*invt), invt = 1/(1+0.1*sigma);  Z = sum(e)
  T  = tanh(u/2), u = (x-m)/(sigma+eps)      (gate g = (1+T)/2)
  sp = 1 + log1p(relu(x-m)) = Ln(E*relu(x-m) + E)
  th = tanh(x/sp) with 1/sp ~ C0*bitcast(~bits(sp)): fp16 exponent-
       complement reciprocal whose Chebyshev constant C0 folds into the
       Tanh scale input (2 cheap DVE ops replace an 8-op Newton chain;
       ~6% sawtooth on the tanh argument -> ~7e-5 relative on the final
       scalar loss, tolerance is 2e-2)
  lg = ln(g) = Ln(0.5*T + 0.5)
  sums: Z, ST, Sth, Slg ride free ACT accum_out; TTH/TTE/TTL = sum(T*y)
       as single fused scalar_tensor_tensor ops with accum_out
  Sc = 0.125*(V + ST + (Sth+TTH)) + 0.75 - 0.25*(Z+TTE)/Z... assembled in
       finalize as Sc = 0.125*(V+ST+P1) + 1 - 0.25*P2/Z with
       P1 = Sth+TTH, P2 = Z+TTE; loss_row = -ln(clip(c_t/(Sc+eps),eps,1))
       + 0.005*(Slg+TTL)

Engine plan: ACT does the 5 transcendental passes (Exp/Ln/Tanh/Tanh/Ln)
plus an unchained Square pass for sum(x^2) (square lives in every ACT
table set so it never forces a table load); DVE does the f32->fp16 cast
(with sum(x) accum), relu(x-m), the 2-op reciprocal trick, t2 = x*bc and
the three fused product-accumulates. Table sets cycle
{natural_log_exp: sp, lg} -> {exp_and_others: e, T, th} once per group of
8-9 chunks; lg of group k runs in the NLE phase of group k+1. Row sqrt for
sigma uses an int32 rsqrt bit-trick on DVE so the sqrt table set never
enters the ACT chain. x is staged to DRAM as fp16 once and re-read per
phase (cheaper than holding 2G+1 chunk tiles in SBUF). The optional
repeats= argument replicates the whole body inside one NEFF for
steady-state timing; the graded entry point uses repeats=1.
"""

import numpy as np

import concourse.bass as bass
import concourse.tile as tile
from concourse import mybir
from concourse.tile import add_dep_helper

# fp16 exponent-complement reciprocal: bitcast(~bits(x)) lands x*bc in
# [-4.5, -4] (same interval as the fp32 trick in dve_ops.py); one
# Chebyshev-scaled Newton step gives ~0.2% rel err.
RECIP_C0 = -0.23549792
RECIP_C1 = 2.0017324


def _split_multi_waits(nc):
    """This walrus build rejects instructions carrying more than one sync
    wait. Hoist extra waits onto same-engine no-ops placed just before."""
    n_split = [0]
    for fn in nc.m.functions:
        for bb in fn.blocks:
            out = []
            for inst in bb.instructions:
                si = inst.sync_info
                waits = list(si.on_wait) if (si is not None and si.on_wait) else []
                if len(waits) > 1:
                    for w in waits[:-1]:
                        n_split[0] += 1
                        nop = mybir.InstNoOp(
                            name=f"waitsplit_{n_split[0]}",
                            engine=inst.engine,
                            bass_nofuse=True,
                        )
                        nop.sync_info = mybir.SyncInfo(on_wait=[w], on_update=[])
                        out.append(nop)
                    inst.sync_info = mybir.SyncInfo(
                        on_wait=[waits[-1]], on_update=list(si.on_update or []))
                out.append(inst)
            bb.instructions[:] = out
    return n_split[0]


F32 = mybir.dt.float32
F16 = mybir.dt.float16
ALU = mybir.AluOpType
ACTF = mybir.ActivationFunctionType

V = 50257
B, S = 4, 512
NROWS = B * S
NCORES = 8
ROWS_PER_CORE = NROWS // NCORES   # 256
P = 128
NT = ROWS_PER_CORE // P           # 2 row-tiles per core
F = 2048                          # vocab chunk
NCHUNK = (V + F - 1) // F         # 25
CHUNKS = [(j * F, min(F, V - j * F)) for j in range(NCHUNK)]
GROUPS = [list(range(0, 9)), list(range(9, 17)), list(range(17, 25))]

ALPHA = 0.5
BETA = 0.1
EPS = 1e-10
E_CONST = float(np.e)


def build_kernel(tc, x, xt, x16, out, repeats=1):
    nc = tc.nc

    act_chain = [None]

    def chain(instr):
        # Serialize ACT in issue order so activations stay grouped by table
        # set (the scheduler is otherwise free to interleave exp/ln/tanh).
        if act_chain[0] is not None:
            add_dep_helper(instr.ins, act_chain[0].ins, False,
                           "ACT table-set ordering")
        act_chain[0] = instr
        return instr

    from contextlib import ExitStack
    with ExitStack() as ctx:
        stg = ctx.enter_context(tc.tile_pool(name="stg", bufs=2))
        cvp = ctx.enter_context(tc.tile_pool(name="cvp", bufs=2))
        xa = ctx.enter_context(tc.tile_pool(name="xa", bufs=4))
        xb = ctx.enter_context(tc.tile_pool(name="xb", bufs=4))
        wp = ctx.enter_context(tc.tile_pool(name="wp", bufs=2))
        spp = ctx.enter_context(tc.tile_pool(name="spp", bufs=2))
        rcp = ctx.enter_context(tc.tile_pool(name="rcp", bufs=2))
        t2p = ctx.enter_context(
            tc.tile_pool(name="t2p", bufs=len(GROUPS[0]) + 1))
        ep = ctx.enter_context(tc.tile_pool(name="ep", bufs=2))
        Tp = ctx.enter_context(tc.tile_pool(name="Tp", bufs=len(GROUPS[0]) + 2))
        thp = ctx.enter_context(tc.tile_pool(name="thp", bufs=2))
        lgp = ctx.enter_context(tc.tile_pool(name="lgp", bufs=2))
        dmp = ctx.enter_context(tc.tile_pool(name="dmp", bufs=2))
        pdm = ctx.enter_context(tc.tile_pool(name="pdm", bufs=2))
        sing = ctx.enter_context(tc.tile_pool(name="sing", bufs=1))

        rep_box = [0]

        # persistent per-row stats, one column per row-tile
        def s2(tag):
            return sing.tile([P, NT], F32, tag=tag, name=tag)

        m2, var2, sig2 = s2("m2"), s2("var2"), s2("sig2")
        invt2, istd22, nb22 = s2("invt2"), s2("istd22"), s2("nb22")
        QN = ["Z", "ST", "Sth", "Slg", "TTH", "TTE", "TTL"]
        sums = {q: s2("sum_" + q) for q in QN}
        Sx2, Sxx2 = s2("Sx2"), s2("Sxx2")

        cE = sing.tile([P, 1], F32, tag="cE", name="cE")
        nc.vector.memset(cE, E_CONST)
        cHalf = sing.tile([P, 1], F32, tag="cHalf", name="cHalf")
        nc.vector.memset(cHalf, 0.5)
        # [P,1] AP scalar for stt ops: a float immediate is modeled as a
        # 4-byte operand and knocks fp16 stt from 2x_1P down to 1x mode.
        cOne = sing.tile([P, 1], F32, tag="cOne", name="cOne")
        nc.vector.memset(cOne, 1.0)

        partials = {}

        def make_partials():
            if partials:
                return
            for t in range(NT):
                for q in QN + ["Sx", "Sxx"]:
                    partials[(q, t)] = sing.tile(
                        [P, NCHUNK], F32, tag=f"p_{q}_{t}",
                        name=f"p_{q}_{t}")

        def pass1(t, jlist):
            for j in jlist:
                c0, cs = CHUNKS[j]
                sg = stg.tile([P, F], F32, tag="stg")
                nc.default_dma_engine.dma_start(
                    out=sg[:, :cs], in_=x[t, :, c0:c0 + cs])
                cv = cvp.tile([P, F], F16, tag="cv")
                nc.vector.tensor_scalar(
                    out=cv[:, :cs], in0=sg[:, :cs], scalar1=cOne, scalar2=0.0,
                    op0=ALU.mult, op1=ALU.add,
                    accum_out=partials[("Sx", t)][:, j:j + 1])
                dm = pdm.tile([P, F], F16, tag="pdm")
                nc.scalar.activation(
                    out=dm[:, :cs], in_=cv[:, :cs], func=ACTF.Square,
                    accum_out=partials[("Sxx", t)][:, j:j + 1])
                nc.default_dma_engine.dma_start(
                    out=x16[t, :, c0:c0 + cs], in_=cv[:, :cs])

        def stats(t):
            ts = slice(t, t + 1)
            nc.vector.tensor_reduce(
                out=Sx2[:, ts], in_=partials[("Sx", t)],
                axis=mybir.AxisListType.X, op=ALU.add)
            nc.vector.tensor_reduce(
                out=Sxx2[:, ts], in_=partials[("Sxx", t)],
                axis=mybir.AxisListType.X, op=ALU.add)
            nc.vector.tensor_scalar(
                out=m2[:, ts], in0=Sx2[:, ts], scalar1=1.0 / V, scalar2=None,
                op0=ALU.mult)
            # var = (Sxx - Sx*m) / (V-1)  [unbiased]
            nc.vector.scalar_tensor_tensor(
                out=var2[:, ts], in0=Sx2[:, ts], scalar=m2[:, ts],
                in1=Sxx2[:, ts], op0=ALU.mult, op1=ALU.subtract)
            nc.vector.tensor_scalar(
                out=var2[:, ts], in0=var2[:, ts], scalar1=-1.0 / (V - 1),
                scalar2=None, op0=ALU.mult)
            # sig = var * rsqrt(var), rsqrt via int32 magic + 2 Newton steps
            # (keeps Sqrt's table set out of the ACT chain)
            I32 = mybir.dt.int32
            ry = sing.tile([P, NT], F32, tag=f"ry{t}", name=f"ry{t}")
            rt_ = sing.tile([P, NT], F32, tag=f"rt{t}", name=f"rt{t}")
            nc.vector.tensor_scalar(
                out=ry[:, ts].bitcast(I32), in0=var2[:, ts].bitcast(I32),
                scalar1=1, scalar2=None, op0=ALU.logical_shift_right)
            nc.vector.tensor_scalar(
                out=ry[:, ts].bitcast(I32), in0=ry[:, ts].bitcast(I32),
                scalar1=-1, scalar2=0x5F3759DF, op0=ALU.mult, op1=ALU.add)
            for _ in range(2):
                nc.vector.tensor_mul(out=rt_[:, ts], in0=ry[:, ts], in1=ry[:, ts])
                nc.vector.tensor_mul(out=rt_[:, ts], in0=rt_[:, ts], in1=var2[:, ts])
                nc.vector.tensor_scalar(
                    out=rt_[:, ts], in0=rt_[:, ts], scalar1=-0.5, scalar2=1.5,
                    op0=ALU.mult, op1=ALU.add)
                nc.vector.tensor_mul(out=ry[:, ts], in0=ry[:, ts], in1=rt_[:, ts])
            nc.vector.tensor_mul(out=sig2[:, ts], in0=var2[:, ts], in1=ry[:, ts])
            # invt = 1/(1 + 0.1*sigma)
            nc.vector.tensor_scalar(
                out=invt2[:, ts], in0=sig2[:, ts], scalar1=BETA, scalar2=1.0,
                op0=ALU.mult, op1=ALU.add)
            nc.vector.reciprocal(out=invt2[:, ts], in_=invt2[:, ts])
            # istd2 = 1/(2*sigma + 2e-10)
            nc.vector.tensor_scalar(
                out=istd22[:, ts], in0=sig2[:, ts], scalar1=2.0,
                scalar2=2.0 * EPS, op0=ALU.mult, op1=ALU.add)
            nc.vector.reciprocal(out=istd22[:, ts], in_=istd22[:, ts])
            # nb2 = -m * istd2
            nc.vector.tensor_scalar(
                out=nb22[:, ts], in0=m2[:, ts], scalar1=istd22[:, ts],
                scalar2=-1.0, op0=ALU.mult, op1=ALU.mult)

        def nle_phase(t, jlist, pend):
            """Ln set: sp for this group's chunks, lg+TTL for previous group."""
            ts = slice(t, t + 1)
            sps = {}
            I16 = mybir.dt.int16
            xcs = {}
            for j in jlist:
                c0, cs = CHUNKS[j]
                xc = xa.tile([P, F], F16, tag="xa")
                nc.default_dma_engine.dma_start(
                    out=xc[:, :cs], in_=x16[t, :, c0:c0 + cs])
                xcs[j] = xc
            for j in jlist:
                c0, cs = CHUNKS[j]
                xc = xcs[j]
                w = wp.tile([P, F], F16, tag="w")
                nc.vector.tensor_scalar(
                    out=w[:, :cs], in0=xc[:, :cs], scalar1=m2[:, ts],
                    scalar2=0.0, op0=ALU.subtract, op1=ALU.max)
                sp = spp.tile([P, F], F16, tag="sp")
                chain(nc.scalar.activation(
                    out=sp[:, :cs], in_=w[:, :cs],
                    func=ACTF.Ln, scale=E_CONST, bias=cE))
                sps[j] = (xc, sp)
            for (j, Tprev, csp) in pend:
                lg = lgp.tile([P, F], F16, tag="lg")
                chain(nc.scalar.activation(
                    out=lg[:, :csp], in_=Tprev[:, :csp],
                    func=ACTF.Ln, scale=0.5, bias=cHalf,
                    accum_out=partials[("Slg", t)][:, j:j + 1]))
                dm = dmp.tile([P, F], F16, tag="dm")
                nc.vector.scalar_tensor_tensor(
                    out=dm[:, :csp], in0=Tprev[:, :csp], scalar=cOne,
                    in1=lg[:, :csp], op0=ALU.mult, op1=ALU.mult,
                    accum_out=partials[("TTL", t)][:, j:j + 1])
            t2s = {}
            for j in jlist:
                c0, cs = CHUNKS[j]
                xc, sp = sps[j]
                # 1/sp ~ C0*bitcast(~bits(sp)); C0 is folded into the
                # downstream Tanh's scale, so just: t2 = x * bitcast(~sp)
                bc = rcp.tile([P, F], F16, tag="bc")
                nc.vector.tensor_scalar(
                    out=bc[:, :cs].bitcast(I16), in0=sp[:, :cs].bitcast(I16),
                    scalar1=-1, scalar2=None, op0=ALU.bitwise_xor)
                t2 = t2p.tile([P, F], F16, tag="t2")
                nc.vector.tensor_mul(
                    out=t2[:, :cs], in0=bc[:, :cs], in1=xc[:, :cs])
                t2s[j] = t2
            return t2s

        def exp_phase(t, jlist, t2s):
            """exp_and_others set: e, T, th + P1/P2 products."""
            ts = slice(t, t + 1)
            pend = []
            xcs = {}
            for j in jlist:
                c0, cs = CHUNKS[j]
                xc = xb.tile([P, F], F16, tag="xb")
                nc.default_dma_engine.dma_start(
                    out=xc[:, :cs], in_=x16[t, :, c0:c0 + cs])
                xcs[j] = xc
            for j in jlist:
                c0, cs = CHUNKS[j]
                xc = xcs[j]
                e = ep.tile([P, F], F16, tag="e")
                chain(nc.scalar.activation(
                    out=e[:, :cs], in_=xc[:, :cs], func=ACTF.Exp,
                    scale=invt2[:, ts],
                    accum_out=partials[("Z", t)][:, j:j + 1]))
                T = Tp.tile([P, F], F16, tag="T")
                chain(nc.scalar.activation(
                    out=T[:, :cs], in_=xc[:, :cs], func=ACTF.Tanh,
                    scale=istd22[:, ts], bias=nb22[:, ts],
                    accum_out=partials[("ST", t)][:, j:j + 1]))
                th = thp.tile([P, F], F16, tag="th")
                chain(nc.scalar.activation(
                    out=th[:, :cs], in_=t2s[j][:, :cs], func=ACTF.Tanh,
                    scale=RECIP_C0,
                    accum_out=partials[("Sth", t)][:, j:j + 1]))
                dm1 = dmp.tile([P, F], F16, tag="dm")
                nc.vector.scalar_tensor_tensor(
                    out=dm1[:, :cs], in0=T[:, :cs], scalar=cOne,
                    in1=th[:, :cs], op0=ALU.mult, op1=ALU.mult,
                    accum_out=partials[("TTH", t)][:, j:j + 1])
                dm2 = dmp.tile([P, F], F16, tag="dm")
                nc.vector.scalar_tensor_tensor(
                    out=dm2[:, :cs], in0=T[:, :cs], scalar=cOne,
                    in1=e[:, :cs], op0=ALU.mult, op1=ALU.mult,
                    accum_out=partials[("TTE", t)][:, j:j + 1])
                pend.append((j, T, cs))
            return pend

        def flush_lg(t, pend):
            for (j, Tprev, csp) in pend:
                lg = lgp.tile([P, F], F16, tag="lg")
                chain(nc.scalar.activation(
                    out=lg[:, :csp], in_=Tprev[:, :csp],
                    func=ACTF.Ln, scale=0.5, bias=cHalf,
                    accum_out=partials[("Slg", t)][:, j:j + 1]))
                dm = dmp.tile([P, F], F16, tag="dm")
                nc.vector.scalar_tensor_tensor(
                    out=dm[:, :csp], in0=Tprev[:, :csp], scalar=cOne,
                    in1=lg[:, :csp], op0=ALU.mult, op1=ALU.mult,
                    accum_out=partials[("TTL", t)][:, j:j + 1])

        def passC(t, pass1_cb=None):
            pend = []
            n_phases = 2 * len(GROUPS) + 1
            done = [0]

            def maybe_pass1():
                if pass1_cb is not None:
                    pass1_cb(done[0], n_phases)
                    done[0] += 1

            for jlist in GROUPS:
                t2s = nle_phase(t, jlist, pend)
                maybe_pass1()
                pend = exp_phase(t, jlist, t2s)
                maybe_pass1()
            flush_lg(t, pend)
            maybe_pass1()

        def finalize():
            for t in range(NT):
                for q in QN:
                    nc.vector.tensor_reduce(
                        out=sums[q][:, t:t + 1], in_=partials[(q, t)],
                        axis=mybir.AxisListType.X, op=ALU.add)

            def tmp(tag):
                return sing.tile([P, NT], F32, tag=tag, name=tag)

            xts = tmp("xts")
            nc.default_dma_engine.dma_start(out=xts, in_=xt)

            Z, ST = sums["Z"], sums["ST"]
            # P1s = sum((1+T)th) = Sth + TTH; P2s = sum((1+T)e) = Z + TTE;
            # P3s = sum((1+T)lg) = Slg + TTL
            P1s, P2s, P3s = s2("P1s"), s2("P2s"), s2("P3s")
            nc.vector.tensor_add(out=P1s, in0=sums["Sth"], in1=sums["TTH"])
            nc.vector.tensor_add(out=P2s, in0=Z, in1=sums["TTE"])
            nc.vector.tensor_add(out=P3s, in0=sums["Slg"], in1=sums["TTL"])
            # target-row pieces (sp_t via NLE set: issue right after flush_lg)
            wt, spt = tmp("wt"), tmp("spt")
            nc.vector.tensor_sub(out=wt, in0=xts, in1=m2)
            nc.vector.tensor_scalar(
                out=wt, in0=wt, scalar1=0.0, scalar2=None, op0=ALU.max)
            chain(nc.scalar.activation(
                out=spt, in_=wt, func=ACTF.Ln, scale=E_CONST, bias=cE))
            rct, t2t = tmp("rct"), tmp("t2t")
            nc.vector.reciprocal(out=rct, in_=spt)
            nc.vector.tensor_mul(out=t2t, in0=xts, in1=rct)
            # [exp_and_others] T_t, th_t, e_t
            ut = tmp("ut")
            nc.vector.tensor_mul(out=ut, in0=xts, in1=istd22)
            nc.vector.tensor_add(out=ut, in0=ut, in1=nb22)
            Tt, tht, et, v1t = tmp("Tt"), tmp("tht"), tmp("et"), tmp("v1t")
            chain(nc.scalar.activation(out=Tt, in_=ut, func=ACTF.Tanh))
            chain(nc.scalar.activation(out=tht, in_=t2t, func=ACTF.Tanh))
            nc.vector.tensor_mul(out=v1t, in0=xts, in1=invt2)
            chain(nc.scalar.activation(out=et, in_=v1t, func=ACTF.Exp))
            gt, rt = tmp("gt"), tmp("rt")
            nc.vector.tensor_scalar(
                out=gt, in0=Tt, scalar1=0.5, scalar2=0.5, op0=ALU.mult,
                op1=ALU.add)
            nc.vector.tensor_scalar(
                out=rt, in0=tht, scalar1=0.5, scalar2=0.5, op0=ALU.mult,
                op1=ALU.add)
            # ct = g*0.5*(r - e/Z) + e/Z
            rZ, erz, d1, h1, ct = tmp("rZ"), tmp("erz"), tmp("d1"), tmp("h1"), tmp("ct")
            nc.vector.reciprocal(out=rZ, in_=Z)
            nc.vector.tensor_mul(out=erz, in0=et, in1=rZ)
            nc.vector.tensor_sub(out=d1, in0=rt, in1=erz)
            nc.vector.tensor_mul(out=h1, in0=gt, in1=d1)
            nc.vector.scalar_tensor_tensor(
                out=ct, in0=h1, scalar=0.5, in1=erz, op0=ALU.mult, op1=ALU.add)
            # Sc = 0.125*(V + ST + P1s) + 1 - 0.25*P2s/Z
            a1, s1, q1, Sc = tmp("a1"), tmp("s1"), tmp("q1"), tmp("Sc")
            nc.vector.tensor_add(out=a1, in0=ST, in1=P1s)
            nc.vector.tensor_scalar(
                out=s1, in0=a1, scalar1=0.125, scalar2=0.125 * V + 1.0,
                op0=ALU.mult, op1=ALU.add)
            nc.vector.tensor_mul(out=q1, in0=P2s, in1=rZ)
            nc.vector.scalar_tensor_tensor(
                out=Sc, in0=q1, scalar=-0.25, in1=s1, op0=ALU.mult, op1=ALU.add)
            scd, rsc, pt = tmp("scd"), tmp("rsc"), tmp("pt")
            nc.vector.tensor_scalar(
                out=scd, in0=Sc, scalar1=EPS, scalar2=None, op0=ALU.add)
            nc.vector.reciprocal(out=rsc, in_=scd)
            nc.vector.tensor_mul(out=pt, in0=ct, in1=rsc)
            nc.vector.tensor_scalar(
                out=pt, in0=pt, scalar1=EPS, scalar2=1.0, op0=ALU.max,
                op1=ALU.min)
            lnp = tmp("lnp")
            chain(nc.scalar.activation(out=lnp, in_=pt, func=ACTF.Ln))
            loss = tmp("loss")
            nc.vector.scalar_tensor_tensor(
                out=loss, in0=P3s, scalar=0.005, in1=lnp, op0=ALU.mult,
                op1=ALU.subtract)
            nc.default_dma_engine.dma_start(out=out, in_=loss)

        # schedule: pass1(0); stats(0); passC(0) with pass1(1) interleaved
        # between phases; stats(1); passC(1); finalize.
        def p1cb(i, n):
            per = (NCHUNK + n - 1) // n
            jl = list(range(i * per, min((i + 1) * per, NCHUNK)))
            if jl:
                pass1(1, jl)

        for rep in range(repeats):
            rep_box[0] = rep
            make_partials()
            pass1(0, list(range(NCHUNK)))
            stats(0)
            passC(0, pass1_cb=p1cb)
            stats(1)
            passC(1)
            finalize()


def build_nc(split_waits=True, repeats=1):
    nc = bass.Bass("TRN2", debug=False, target_bir_lowering=False,
                   num_devices=NCORES)
    x = nc.dram_tensor("x", [NT, P, V], F32, kind="ExternalInput").ap()
    xt = nc.dram_tensor("xt", [P, NT], F32, kind="ExternalInput").ap()
    x16 = nc.dram_tensor("x16", [NT, P, V], F16).ap()
    out = nc.dram_tensor("out", [P, NT], F32, kind="ExternalOutput").ap()
    with tile.TileContext(nc) as tc:
        build_kernel(tc, x, xt, x16, out, repeats=repeats)
    if split_waits:
        _split_multi_waits(nc)
    return nc


_NC_CACHE = None


def _get_nc():
    global _NC_CACHE
    if _NC_CACHE is None:
        _NC_CACHE = build_nc()
    return _NC_CACHE


def make_in_maps(logits, targets):
    lg = np.ascontiguousarray(np.asarray(logits, dtype=np.float32)).reshape(
        NROWS, V)
    tg = np.asarray(targets).reshape(NROWS).astype(np.int64)
    xt_rows = lg[np.arange(NROWS), tg].astype(np.float32)
    in_maps = []
    for c in range(NCORES):
        r0 = c * ROWS_PER_CORE
        x_c = lg[r0:r0 + ROWS_PER_CORE].reshape(NT, P, V)
        xt_c = np.ascontiguousarray(
            xt_rows[r0:r0 + ROWS_PER_CORE].reshape(NT, P).T)
        in_maps.append({"x": x_c, "xt": xt_c})
    return in_maps


def kernel(logits, targets):
    from concourse.bass_utils import run_bass_kernel_spmd
    nc = _get_nc()
    in_maps = make_in_maps(logits, targets)
    res = run_bass_kernel_spmd(nc, in_maps, core_ids=list(range(NCORES)))
    rows = np.concatenate(
        [res.results[c]["out"].T.reshape(ROWS_PER_CORE) for c in range(NCORES)])
    return np.asarray(rows.mean(), dtype=np.float32)


# revision 4
# speedup vs baseline: 2.3850x; 1.0006x over previous
"""Trainium2 Bass kernel for nn_AdaptiveGatingHybridActivation.

Data-parallel across 8 NeuronCores: each core does 256 rows (2 tiles of 128
partitions) x V=50257 vocab. Per-row math (x row, m/sigma mean/unbiased-std):
  e  = exp(x*invt), invt = 1/(1+0.1*sigma);  Z = sum(e)
  T  = tanh(u/2), u = (x-m)/(sigma+eps)      (gate g = (1+T)/2)
  sp = 1 + log1p(relu(x-m)) = Ln(E*relu(x-m) + E)
  th = tanh(x/sp) with 1/sp ~ C0*bitcast(~bits(sp)): fp16 exponent-
       complement reciprocal whose Chebyshev constant C0 folds into the
       Tanh scale input (2 cheap DVE ops replace an 8-op Newton chain;
       ~6% sawtooth on the tanh argument -> ~7e-5 relative on the final
       scalar loss, tolerance is 2e-2)
  lg = ln(g) = Ln(0.5*T + 0.5)
  sums: Z, ST, Sth, Slg ride free ACT accum_out; TTH/TTE/TTL = sum(T*y)
       as single fused scalar_tensor_tensor ops with accum_out
  Sc = 0.125*(V + ST + (Sth+TTH)) + 0.75 - 0.25*(Z+TTE)/Z... assembled in
       finalize as Sc = 0.125*(V+ST+P1) + 1 - 0.25*P2/Z with
       P1 = Sth+TTH, P2 = Z+TTE; loss_row = -ln(clip(c_t/(Sc+eps),eps,1))
       + 0.005*(Slg+TTL)

Engine plan: ACT does the 5 transcendental passes (Exp/Ln/Tanh/Tanh/Ln)
plus an unchained Square pass for sum(x^2) (square lives in every ACT
table set so it never forces a table load); DVE does the f32->fp16 cast
(with sum(x) accum), relu(x-m), the 2-op reciprocal trick, t2 = x*bc and
the three fused product-accumulates. Table sets cycle
{natural_log_exp: sp, lg} -> {exp_and_others: e, T, th} once per group of
8-9 chunks; lg of group k runs in the NLE phase of group k+1. Row sqrt for
sigma uses an int32 rsqrt bit-trick on DVE so the sqrt table set never
enters the ACT chain. x is staged to DRAM as fp16 once and re-read per
phase (cheaper than holding 2G+1 chunk tiles in SBUF). The optional
repeats= argument replicates the whole body inside one NEFF for
steady-state timing; the graded entry point uses repeats=1.
"""

import numpy as np

import concourse.bass as bass
import concourse.tile as tile
from concourse import mybir
from concourse.tile import add_dep_helper

# fp16 exponent-complement reciprocal: bitcast(~bits(x)) lands x*bc in
# [-4.5, -4] (same interval as the fp32 trick in dve_ops.py); one
# Chebyshev-scaled Newton step gives ~0.2% rel err.
RECIP_C0 = -0.23549792
RECIP_C1 = 2.0017324


def _split_multi_waits(nc):
    """This walrus build rejects instructions carrying more than one sync
    wait. Hoist extra waits onto same-engine no-ops placed just before."""
    n_split = [0]
    for fn in nc.m.functions:
        for bb in fn.blocks:
            out = []
            for inst in bb.instructions:
                si = inst.sync_info
                waits = list(si.on_wait) if (si is not None and si.on_wait) else []
                if len(waits) > 1:
                    for w in waits[:-1]:
                        n_split[0] += 1
                        nop = mybir.InstNoOp(
                            name=f"waitsplit_{n_split[0]}",
                            engine=inst.engine,
                            bass_nofuse=True,
                        )
                        nop.sync_info = mybir.SyncInfo(on_wait=[w], on_update=[])
                        out.append(nop)
                    inst.sync_info = mybir.SyncInfo(
                        on_wait=[waits[-1]], on_update=list(si.on_update or []))
                out.append(inst)
            bb.instructions[:] = out
    return n_split[0]


F32 = mybir.dt.float32
F16 = mybir.dt.float16
ALU = mybir.AluOpType
ACTF = mybir.ActivationFunctionType

V = 50257
B, S = 4, 512
NROWS = B * S
NCORES = 8
ROWS_PER_CORE = NROWS // NCORES   # 256
P = 128
NT = ROWS_PER_CORE // P           # 2 row-tiles per core
F = 2048                          # vocab chunk
NCHUNK = (V + F - 1) // F         # 25
CHUNKS = [(j * F, min(F, V - j * F)) for j in range(NCHUNK)]
GROUPS = [list(range(0, 9)), list(range(9, 17)), list(range(17, 25))]

ALPHA = 0.5
BETA = 0.1
EPS = 1e-10
E_CONST = float(np.e)


def build_kernel(tc, x, xt, x16, out, repeats=1):
    nc = tc.nc

    act_chain = [None]

    def chain(instr):
        # Serialize ACT in issue order so activations stay grouped by table
        # set (the scheduler is otherwise free to interleave exp/ln/tanh).
        if act_chain[0] is not None:
            add_dep_helper(instr.ins, act_chain[0].ins, False,
                           "ACT table-set ordering")
        act_chain[0] = instr
        return instr

    from contextlib import ExitStack
    with ExitStack() as ctx:
        stg = ctx.enter_context(tc.tile_pool(name="stg", bufs=2))
        cvp = ctx.enter_context(tc.tile_pool(name="cvp", bufs=2))
        xa = ctx.enter_context(tc.tile_pool(name="xa", bufs=4))
        xb = ctx.enter_context(tc.tile_pool(name="xb", bufs=4))
        wp = ctx.enter_context(tc.tile_pool(name="wp", bufs=2))
        spp = ctx.enter_context(tc.tile_pool(name="spp", bufs=2))
        rcp = ctx.enter_context(tc.tile_pool(name="rcp", bufs=2))
        t2p = ctx.enter_context(
            tc.tile_pool(name="t2p", bufs=len(GROUPS[0]) + 1))
        ep = ctx.enter_context(tc.tile_pool(name="ep", bufs=2))
        Tp = ctx.enter_context(tc.tile_pool(name="Tp", bufs=len(GROUPS[0]) + 2))
        thp = ctx.enter_context(tc.tile_pool(name="thp", bufs=2))
        lgp = ctx.enter_context(tc.tile_pool(name="lgp", bufs=2))
        dmp = ctx.enter_context(tc.tile_pool(name="dmp", bufs=2))
        pdm = ctx.enter_context(tc.tile_pool(name="pdm", bufs=2))
        sing = ctx.enter_context(tc.tile_pool(name="sing", bufs=1))

        rep_box = [0]

        # persistent per-row stats, one column per row-tile
        def s2(tag):
            return sing.tile([P, NT], F32, tag=tag, name=tag)

        m2, var2, sig2 = s2("m2"), s2("var2"), s2("sig2")
        invt2, istd22, nb22 = s2("invt2"), s2("istd22"), s2("nb22")
        QN = ["Z", "ST", "Sth", "Slg", "TTH", "TTE", "TTL"]
        sums = {q: s2("sum_" + q) for q in QN}
        Sx2, Sxx2 = s2("Sx2"), s2("Sxx2")

        cE = sing.tile([P, 1], F32, tag="cE", name="cE")
        nc.vector.memset(cE, E_CONST)
        cHalf = sing.tile([P, 1], F32, tag="cHalf", name="cHalf")
        nc.vector.memset(cHalf, 0.5)
        # [P,1] AP scalar for stt ops: a float immediate is modeled as a
        # 4-byte operand and knocks fp16 stt from 2x_1P down to 1x mode.
        cOne = sing.tile([P, 1], F32, tag="cOne", name="cOne")
        nc.vector.memset(cOne, 1.0)

        partials = {}

        def make_partials():
            if partials:
                return
            for t in range(NT):
                for q in QN + ["Sx", "Sxx"]:
                    partials[(q, t)] = sing.tile(
                        [P, NCHUNK], F32, tag=f"p_{q}_{t}",
                        name=f"p_{q}_{t}")

        def pass1(t, jlist):
            for j in jlist:
                c0, cs = CHUNKS[j]
                sg = stg.tile([P, F], F32, tag="stg")
                nc.default_dma_engine.dma_start(
                    out=sg[:, :cs], in_=x[t, :, c0:c0 + cs])
                cv = cvp.tile([P, F], F16, tag="cv")
                nc.vector.tensor_scalar(
                    out=cv[:, :cs], in0=sg[:, :cs], scalar1=cOne, scalar2=0.0,
                    op0=ALU.mult, op1=ALU.add,
                    accum_out=partials[("Sx", t)][:, j:j + 1])
                dm = pdm.tile([P, F], F16, tag="pdm")
                nc.scalar.activation(
                    out=dm[:, :cs], in_=cv[:, :cs], func=ACTF.Square,
                    accum_out=partials[("Sxx", t)][:, j:j + 1])
                nc.default_dma_engine.dma_start(
                    out=x16[t, :, c0:c0 + cs], in_=cv[:, :cs])

        def stats(t):
            ts = slice(t, t + 1)
            nc.vector.tensor_reduce(
                out=Sx2[:, ts], in_=partials[("Sx", t)],
                axis=mybir.AxisListType.X, op=ALU.add)
            nc.vector.tensor_reduce(
                out=Sxx2[:, ts], in_=partials[("Sxx", t)],
                axis=mybir.AxisListType.X, op=ALU.add)
            nc.vector.tensor_scalar(
                out=m2[:, ts], in0=Sx2[:, ts], scalar1=1.0 / V, scalar2=None,
                op0=ALU.mult)
            # var = (Sxx - Sx*m) / (V-1)  [unbiased]
            nc.vector.scalar_tensor_tensor(
                out=var2[:, ts], in0=Sx2[:, ts], scalar=m2[:, ts],
                in1=Sxx2[:, ts], op0=ALU.mult, op1=ALU.subtract)
            nc.vector.tensor_scalar(
                out=var2[:, ts], in0=var2[:, ts], scalar1=-1.0 / (V - 1),
                scalar2=None, op0=ALU.mult)
            # sig = var * rsqrt(var), rsqrt via int32 magic + 2 Newton steps
            # (keeps Sqrt's table set out of the ACT chain)
            I32 = mybir.dt.int32
            ry = sing.tile([P, NT], F32, tag=f"ry{t}", name=f"ry{t}")
            rt_ = sing.tile([P, NT], F32, tag=f"rt{t}", name=f"rt{t}")
            nc.vector.tensor_scalar(
                out=ry[:, ts].bitcast(I32), in0=var2[:, ts].bitcast(I32),
                scalar1=1, scalar2=None, op0=ALU.logical_shift_right)
            nc.vector.tensor_scalar(
                out=ry[:, ts].bitcast(I32), in0=ry[:, ts].bitcast(I32),
                scalar1=-1, scalar2=0x5F3759DF, op0=ALU.mult, op1=ALU.add)
            for _ in range(2):
                nc.vector.tensor_mul(out=rt_[:, ts], in0=ry[:, ts], in1=ry[:, ts])
                nc.vector.tensor_mul(out=rt_[:, ts], in0=rt_[:, ts], in1=var2[:, ts])
                nc.vector.tensor_scalar(
                    out=rt_[:, ts], in0=rt_[:, ts], scalar1=-0.5, scalar2=1.5,
                    op0=ALU.mult, op1=ALU.add)
                nc.vector.tensor_mul(out=ry[:, ts], in0=ry[:, ts], in1=rt_[:, ts])
            nc.vector.tensor_mul(out=sig2[:, ts], in0=var2[:, ts], in1=ry[:, ts])
            # invt = 1/(1 + 0.1*sigma)
            nc.vector.tensor_scalar(
                out=invt2[:, ts], in0=sig2[:, ts], scalar1=BETA, scalar2=1.0,
                op0=ALU.mult, op1=ALU.add)
            nc.vector.reciprocal(out=invt2[:, ts], in_=invt2[:, ts])
            # istd2 = 1/(2*sigma + 2e-10)
            nc.vector.tensor_scalar(
                out=istd22[:, ts], in0=sig2[:, ts], scalar1=2.0,
                scalar2=2.0 * EPS, op0=ALU.mult, op1=ALU.add)
            nc.vector.reciprocal(out=istd22[:, ts], in_=istd22[:, ts])
            # nb2 = -m * istd2
            nc.vector.tensor_scalar(
                out=nb22[:, ts], in0=m2[:, ts], scalar1=istd22[:, ts],
                scalar2=-1.0, op0=ALU.mult, op1=ALU.mult)

        def nle_phase(t, jlist, pend):
            """Ln set: sp for this group's chunks, lg+TTL for previous group."""
            ts = slice(t, t + 1)
            sps = {}
            I16 = mybir.dt.int16
            xcs = {}
            for j in jlist:
                c0, cs = CHUNKS[j]
                xc = xa.tile([P, F], F16, tag="xa")
                nc.default_dma_engine.dma_start(
                    out=xc[:, :cs], in_=x16[t, :, c0:c0 + cs])
                xcs[j] = xc
            for j in jlist:
                c0, cs = CHUNKS[j]
                xc = xcs[j]
                w = wp.tile([P, F], F16, tag="w")
                nc.vector.tensor_scalar(
                    out=w[:, :cs], in0=xc[:, :cs], scalar1=m2[:, ts],
                    scalar2=0.0, op0=ALU.subtract, op1=ALU.max)
                sp = spp.tile([P, F], F16, tag="sp")
                chain(nc.scalar.activation(
                    out=sp[:, :cs], in_=w[:, :cs],
                    func=ACTF.Ln, scale=E_CONST, bias=cE))
                sps[j] = (xc, sp)
            for (j, Tprev, csp) in pend:
                lg = lgp.tile([P, F], F16, tag="lg")
                chain(nc.scalar.activation(
                    out=lg[:, :csp], in_=Tprev[:, :csp],
                    func=ACTF.Ln, scale=0.5, bias=cHalf,
                    accum_out=partials[("Slg", t)][:, j:j + 1]))
                dm = dmp.tile([P, F], F16, tag="dm")
                nc.vector.scalar_tensor_tensor(
                    out=dm[:, :csp], in0=Tprev[:, :csp], scalar=cOne,
                    in1=lg[:, :csp], op0=ALU.mult, op1=ALU.mult,
                    accum_out=partials[("TTL", t)][:, j:j + 1])
            t2s = {}
            for j in jlist:
                c0, cs = CHUNKS[j]
                xc, sp = sps[j]
                # 1/sp ~ C0*bitcast(~bits(sp)); C0 is folded into the
                # downstream Tanh's scale, so just: t2 = x * bitcast(~sp)
                bc = rcp.tile([P, F], F16, tag="bc")
                nc.vector.tensor_scalar(
                    out=bc[:, :cs].bitcast(I16), in0=sp[:, :cs].bitcast(I16),
                    scalar1=-1, scalar2=None, op0=ALU.bitwise_xor)
                t2 = t2p.tile([P, F], F16, tag="t2")
                nc.vector.tensor_mul(
                    out=t2[:, :cs], in0=bc[:, :cs], in1=xc[:, :cs])
                t2s[j] = t2
            return t2s

        def exp_phase(t, jlist, t2s):
            """exp_and_others set: e, T, th + P1/P2 products."""
            ts = slice(t, t + 1)
            pend = []
            xcs = {}
            for j in jlist:
                c0, cs = CHUNKS[j]
                xc = xb.tile([P, F], F16, tag="xb")
                nc.default_dma_engine.dma_start(
                    out=xc[:, :cs], in_=x16[t, :, c0:c0 + cs])
                xcs[j] = xc
            for j in jlist:
                c0, cs = CHUNKS[j]
                xc = xcs[j]
                e = ep.tile([P, F], F16, tag="e")
                chain(nc.scalar.activation(
                    out=e[:, :cs], in_=xc[:, :cs], func=ACTF.Exp,
                    scale=invt2[:, ts],
                    accum_out=partials[("Z", t)][:, j:j + 1]))
                T = Tp.tile([P, F], F16, tag="T")
                chain(nc.scalar.activation(
                    out=T[:, :cs], in_=xc[:, :cs], func=ACTF.Tanh,
                    scale=istd22[:, ts], bias=nb22[:, ts],
                    accum_out=partials[("ST", t)][:, j:j + 1]))
                th = thp.tile([P, F], F16, tag="th")
                chain(nc.scalar.activation(
                    out=th[:, :cs], in_=t2s[j][:, :cs], func=ACTF.Tanh,
                    scale=RECIP_C0,
                    accum_out=partials[("Sth", t)][:, j:j + 1]))
                dm1 = dmp.tile([P, F], F16, tag="dm")
                nc.vector.scalar_tensor_tensor(
                    out=dm1[:, :cs], in0=T[:, :cs], scalar=cOne,
                    in1=th[:, :cs], op0=ALU.mult, op1=ALU.mult,
                    accum_out=partials[("TTH", t)][:, j:j + 1])
                dm2 = dmp.tile([P, F], F16, tag="dm")
                nc.vector.scalar_tensor_tensor(
                    out=dm2[:, :cs], in0=T[:, :cs], scalar=cOne,
                    in1=e[:, :cs], op0=ALU.mult, op1=ALU.mult,
                    accum_out=partials[("TTE", t)][:, j:j + 1])
                pend.append((j, T, cs))
            return pend

        def flush_lg(t, pend):
            for (j, Tprev, csp) in pend:
                lg = lgp.tile([P, F], F16, tag="lg")
                chain(nc.scalar.activation(
                    out=lg[:, :csp], in_=Tprev[:, :csp],
                    func=ACTF.Ln, scale=0.5, bias=cHalf,
                    accum_out=partials[("Slg", t)][:, j:j + 1]))
                dm = dmp.tile([P, F], F16, tag="dm")
                nc.vector.scalar_tensor_tensor(
                    out=dm[:, :csp], in0=Tprev[:, :csp], scalar=cOne,
                    in1=lg[:, :csp], op0=ALU.mult, op1=ALU.mult,
                    accum_out=partials[("TTL", t)][:, j:j + 1])

        def passC(t, pass1_cb=None):
            pend = []
            n_phases = 2 * len(GROUPS) + 1
            done = [0]

            def maybe_pass1():
                if pass1_cb is not None:
                    pass1_cb(done[0], n_phases)
                    done[0] += 1

            for jlist in GROUPS:
                t2s = nle_phase(t, jlist, pend)
                maybe_pass1()
                pend = exp_phase(t, jlist, t2s)
                maybe_pass1()
            flush_lg(t, pend)
            maybe_pass1()

        def finalize():
            for t in range(NT):
                for q in QN:
                    nc.vector.tensor_reduce(
                        out=sums[q][:, t:t + 1], in_=partials[(q, t)],
                        axis=mybir.AxisListType.X, op=ALU.add)

            def tmp(tag):
                return sing.tile([P, NT], F32, tag=tag, name=tag)

            xts = tmp("xts")
            nc.default_dma_engine.dma_start(out=xts, in_=xt)

            Z, ST = sums["Z"], sums["ST"]
            # P1s = sum((1+T)th) = Sth + TTH; P2s = sum((1+T)e) = Z + TTE;
            # P3s = sum((1+T)lg) = Slg + TTL
            P1s, P2s, P3s = s2("P1s"), s2("P2s"), s2("P3s")
            nc.vector.tensor_add(out=P1s, in0=sums["Sth"], in1=sums["TTH"])
            nc.vector.tensor_add(out=P2s, in0=Z, in1=sums["TTE"])
            nc.vector.tensor_add(out=P3s, in0=sums["Slg"], in1=sums["TTL"])
            # target-row pieces (sp_t via NLE set: issue right after flush_lg)
            wt, spt = tmp("wt"), tmp("spt")
            nc.vector.tensor_sub(out=wt, in0=xts, in1=m2)
            nc.vector.tensor_scalar(
                out=wt, in0=wt, scalar1=0.0, scalar2=None, op0=ALU.max)
            chain(nc.scalar.activation(
                out=spt, in_=wt, func=ACTF.Ln, scale=E_CONST, bias=cE))
            rct, t2t = tmp("rct"), tmp("t2t")
            nc.vector.reciprocal(out=rct, in_=spt)
            nc.vector.tensor_mul(out=t2t, in0=xts, in1=rct)
            # [exp_and_others] T_t, th_t, e_t
            ut = tmp("ut")
            nc.vector.tensor_mul(out=ut, in0=xts, in1=istd22)
            nc.vector.tensor_add(out=ut, in0=ut, in1=nb22)
            Tt, tht, et, v1t = tmp("Tt"), tmp("tht"), tmp("et"), tmp("v1t")
            chain(nc.scalar.activation(out=Tt, in_=ut, func=ACTF.Tanh))
            chain(nc.scalar.activation(out=tht, in_=t2t, func=ACTF.Tanh))
            nc.vector.tensor_mul(out=v1t, in0=xts, in1=invt2)
            chain(nc.scalar.activation(out=et, in_=v1t, func=ACTF.Exp))
            gt, rt = tmp("gt"), tmp("rt")
            nc.vector.tensor_scalar(
                out=gt, in0=Tt, scalar1=0.5, scalar2=0.5, op0=ALU.mult,
                op1=ALU.add)
            nc.vector.tensor_scalar(
                out=rt, in0=tht, scalar1=0.5, scalar2=0.5, op0=ALU.mult,
                op1=ALU.add)
            # ct = g*0.5*(r - e/Z) + e/Z
            rZ, erz, d1, h1, ct = tmp("rZ"), tmp("erz"), tmp("d1"), tmp("h1"), tmp("ct")
            nc.vector.reciprocal(out=rZ, in_=Z)
            nc.vector.tensor_mul(out=erz, in0=et, in1=rZ)
            nc.vector.tensor_sub(out=d1, in0=rt, in1=erz)
            nc.vector.tensor_mul(out=h1, in0=gt, in1=d1)
            nc.vector.scalar_tensor_tensor(
                out=ct, in0=h1, scalar=0.5, in1=erz, op0=ALU.mult, op1=ALU.add)
            # Sc = 0.125*(V + ST + P1s) + 1 - 0.25*P2s/Z
            a1, s1, q1, Sc = tmp("a1"), tmp("s1"), tmp("q1"), tmp("Sc")
            nc.vector.tensor_add(out=a1, in0=ST, in1=P1s)
            nc.vector.tensor_scalar(
                out=s1, in0=a1, scalar1=0.125, scalar2=0.125 * V + 1.0,
                op0=ALU.mult, op1=ALU.add)
            nc.vector.tensor_mul(out=q1, in0=P2s, in1=rZ)
            nc.vector.scalar_tensor_tensor(
                out=Sc, in0=q1, scalar=-0.25, in1=s1, op0=ALU.mult, op1=ALU.add)
            scd, rsc, pt = tmp("scd"), tmp("rsc"), tmp("pt")
            nc.vector.tensor_scalar(
                out=scd, in0=Sc, scalar1=EPS, scalar2=None, op0=ALU.add)
            nc.vector.reciprocal(out=rsc, in_=scd)
            nc.vector.tensor_mul(out=pt, in0=ct, in1=rsc)
            nc.vector.tensor_scalar(
                out=pt, in0=pt, scalar1=EPS, scalar2=1.0, op0=ALU.max,
                op1=ALU.min)
            lnp = tmp("lnp")
            chain(nc.scalar.activation(out=lnp, in_=pt, func=ACTF.Ln))
            loss = tmp("loss")
            nc.vector.scalar_tensor_tensor(
                out=loss, in0=P3s, scalar=0.005, in1=lnp, op0=ALU.mult,
                op1=ALU.subtract)
            nc.default_dma_engine.dma_start(out=out, in_=loss)

        # schedule: pass1(0); stats(0); passC(0) with pass1(1) interleaved
        # between phases; stats(1); passC(1); finalize.
        def p1cb(i, n):
            per = (NCHUNK + n - 1) // n
            jl = list(range(i * per, min((i + 1) * per, NCHUNK)))
            if jl:
                pass1(1, jl)

        for rep in range(repeats):
            rep_box[0] = rep
            make_partials()
            pass1(0, list(range(NCHUNK)))
            stats(0)
            passC(0, pass1_cb=p1cb)
            stats(1)
            passC(1)
            finalize()


def build_nc(split_waits=True, repeats=1):
    nc = bass.Bass("TRN2", debug=False, target_bir_lowering=False,
                   num_devices=NCORES)
    x = nc.dram_tensor("x", [NT, P, V], F32, kind="ExternalInput").ap()
    xt = nc.dram_tensor("xt", [P, NT], F32, kind="ExternalInput").ap()
    x16 = nc.dram_tensor("x16", [NT, P, V], F16).ap()
    out = nc.dram_tensor("out", [P, NT], F32, kind="ExternalOutput").ap()
    with tile.TileContext(nc) as tc:
        build_kernel(tc, x, xt, x16, out, repeats=repeats)
    if split_waits:
        _split_multi_waits(nc)
    return nc


_NC_CACHE = None


def _get_nc():
    global _NC_CACHE
    if _NC_CACHE is None:
        _NC_CACHE = build_nc()
    return _NC_CACHE


def make_in_maps(logits, targets):
    lg = np.ascontiguousarray(np.asarray(logits, dtype=np.float32)).reshape(
        NROWS, V)
    tg = np.asarray(targets).reshape(NROWS).astype(np.int64)
    xt_rows = lg[np.arange(NROWS), tg].astype(np.float32)
    in_maps = []
    for c in range(NCORES):
        r0 = c * ROWS_PER_CORE
        x_c = lg[r0:r0 + ROWS_PER_CORE].reshape(NT, P, V)
        xt_c = np.ascontiguousarray(
            xt_rows[r0:r0 + ROWS_PER_CORE].reshape(NT, P).T)
        in_maps.append({"x": x_c, "xt": xt_c})
    return in_maps


def kernel(logits, targets):
    from concourse.bass_utils import run_bass_kernel_spmd
    nc = _get_nc()
    in_maps = make_in_maps(logits, targets)
    res = run_bass_kernel_spmd(nc, in_maps, core_ids=list(range(NCORES)))
    rows = np.concatenate(
        [res.results[c]["out"].T.reshape(ROWS_PER_CORE) for c in range(NCORES)])
    return np.asarray(rows.mean(), dtype=np.float32)
